# revision 6
# baseline (speedup 1.0000x reference)
"""Causal attention layer on 8 TRN2 NeuronCores, data-parallel over batch.

Per-core problem (batch element n = core id):
    q = query @ Wq.T ; k = key @ Wk.T              (f32r matmuls)
    scores[s,t] = q[s]·k[t]  for t <= s            (f32r)
    attn = softmax(32 * scores)  (the +1 additive mask cancels in softmax;
                                  -inf masking == skipping t > s)
    ctxT[i,s] = sum_t value[t,i] * attn[s,t]       (bf16)
    out[s,:]  = (ctxT.T @ Wv.T) / rowsum           (f32r, normalization folded)

Layouts: qT/kT are built as [D, S] via PE transposes of the inputs and
weights so every matmul contracts along partitions without DMA transposes.
"""
import numpy as np
from contextlib import ExitStack

import concourse.bass as bass
import concourse.tile as tile
from concourse import bacc, mybir
from concourse.bass_utils import run_bass_kernel_spmd
from concourse.masks import make_identity

F32 = mybir.dt.float32
F32R = mybir.dt.float32r
BF16 = mybir.dt.bfloat16

N, S, T, D = 8, 2048, 2048, 1024
P = 128
NSTRIP = S // P          # 16 query strips
TCH = 512                # t-chunk for score matmuls
OC = D // P              # 8 chunks of the projection/feature dim
SCALE = float(np.sqrt(np.float32(D)))  # 32.0
NEG = -1.0e30

# dtype knobs (QK path needs >= f32r precision; see noise_sim.py)
QK_DT = F32R             # q/k projections + scores matmuls
AV_DT = BF16             # attn weights + value contraction
VP_DT = F32R             # final (attn@value) @ Wv.T projection


def _mm(nc, out, lhsT, rhs, dt, **kw):
    nc.tensor.matmul(out, lhsT.bitcast(dt), rhs.bitcast(dt), **kw)


def build_nc():
    nc = bacc.Bacc("TRN2", target_bir_lowering=False, debug=False,
                   enable_asserts=False)
    q_d = nc.dram_tensor("query", [S, D], F32, kind="ExternalInput")
    k_d = nc.dram_tensor("key", [T, D], F32, kind="ExternalInput")
    v_d = nc.dram_tensor("value", [T, D], F32, kind="ExternalInput")
    wq_d = nc.dram_tensor("Wq", [D, D], F32, kind="ExternalInput")
    wk_d = nc.dram_tensor("Wk", [D, D], F32, kind="ExternalInput")
    wv_d = nc.dram_tensor("Wv", [D, D], F32, kind="ExternalInput")
    out_d = nc.dram_tensor("out", [S, D], F32, kind="ExternalOutput")
    qt_d = nc.dram_tensor("qt_scratch", [D, S], QK_DT)  # internal scratch

    with tile.TileContext(nc) as tc, ExitStack() as ctx:
        const = ctx.enter_context(tc.tile_pool(name="const", bufs=1))
        wt_pool = ctx.enter_context(tc.tile_pool(name="wt", bufs=1))
        kt_pool = ctx.enter_context(tc.tile_pool(name="kt", bufs=1))
        val_pool = ctx.enter_context(tc.tile_pool(name="val", bufs=1))
        stage = ctx.enter_context(tc.tile_pool(name="stage", bufs=2))
        int_pool = ctx.enter_context(tc.tile_pool(name="inT", bufs=2))
        qts_pool = ctx.enter_context(tc.tile_pool(name="qts", bufs=2))
        qstg_pool = ctx.enter_context(tc.tile_pool(name="qstg", bufs=1))
        sc_pool = ctx.enter_context(tc.tile_pool(name="scores", bufs=1))
        exp_pool = ctx.enter_context(tc.tile_pool(name="exp", bufs=2))
        at_pool = ctx.enter_context(tc.tile_pool(name="attnT", bufs=2))
        ctx_pool = ctx.enter_context(tc.tile_pool(name="ctxsb", bufs=2))
        ob_pool = ctx.enter_context(tc.tile_pool(name="outb", bufs=2))
        st_pool = ctx.enter_context(tc.tile_pool(name="stats", bufs=32))
        mm_ps = ctx.enter_context(tc.tile_pool(name="mmps", bufs=4, space="PSUM"))
        ctx_ps = ctx.enter_context(tc.tile_pool(name="ctxps", bufs=2, space="PSUM"))

        ident = const.tile([P, P], F32)
        make_identity(nc, ident)
        ident_bf = const.tile([P, P], BF16)
        nc.vector.tensor_copy(ident_bf[:], ident[:])

        # Additive diag masks for the 4 possible strip positions inside a
        # 512-wide t-chunk: mask[j][x, y] = 0 if y <= j*128 + x else -1e30
        diagmask = const.tile([P, 4, TCH], F32)
        nc.gpsimd.memset(diagmask[:], 0.0)
        for j in range(4):
            nc.gpsimd.affine_select(
                out=diagmask[:, j, :], in_=diagmask[:, j, :],
                compare_op=mybir.AluOpType.is_ge, fill=NEG,
                base=j * P, channel_multiplier=1, pattern=[[-1, TCH]])

        def load_wT(w_dram, dt):
            """[D,D] weight [o,i] -> SBUF [128(i_loc), OC(i_chunk), D(o)]."""
            wt = wt_pool.tile([P, OC, D], dt, name="wT", tag="wT")
            for r in range(OC):              # o-strip
                stg = stage.tile([P, D], F32, name="stage", tag="stage")
                nc.sync.dma_start(stg[:], w_dram.ap()[r * P:(r + 1) * P, :])
                for g in range(0, OC, 4):
                    ps = mm_ps.tile([P, TCH], F32, name="tps", tag="mmps")
                    for c in range(4):
                        nc.tensor.transpose(
                            ps[:, c * P:(c + 1) * P],
                            stg[:, (g + c) * P:(g + c + 1) * P], ident[:])
                    nc.vector.tensor_copy(
                        wt[:, g:g + 4, r * P:(r + 1) * P],
                        ps.rearrange("p (c s) -> p c s", c=4))
            return wt

        def in_transpose_chunk(x_dram, c0, width):
            """x[c0:c0+width, :D] -> xT tile [128(i_loc), OC, width]."""
            xt = int_pool.tile([P, OC, 2 * P], QK_DT, name="inT", tag="inT")
            for sl in range(width // P):
                stg = stage.tile([P, D], F32, name="stage", tag="stage")
                nc.sync.dma_start(
                    stg[:], x_dram.ap()[c0 + sl * P:c0 + (sl + 1) * P, :])
                for g in range(0, OC, 4):
                    ps = mm_ps.tile([P, TCH], F32, name="tps", tag="mmps")
                    for c in range(4):
                        nc.tensor.transpose(
                            ps[:, c * P:(c + 1) * P],
                            stg[:, (g + c) * P:(g + c + 1) * P], ident[:])
                    nc.vector.tensor_copy(
                        xt[:, g:g + 4, sl * P:(sl + 1) * P],
                        ps.rearrange("p (c s) -> p c s", c=4))
            return xt[:, :, :width]

        # ---- phase Q: qT = Wq @ query^T -> DRAM scratch [D, S] ----
        wqT = load_wT(wq_d, QK_DT)
        CW = 256                      # projection chunk width (f32r wants >=256)
        for sc in range(S // CW):
            qT = in_transpose_chunk(q_d, sc * CW, CW)
            for oc in range(OC):
                ps = mm_ps.tile([P, TCH], F32, name="pjps", tag="mmps")
                for ic in range(OC):
                    _mm(nc, ps[:, :CW], wqT[:, ic, oc * P:(oc + 1) * P],
                        qT[:, ic, :], QK_DT,
                        start=(ic == 0), stop=(ic == OC - 1))
                stg = qstg_pool.tile([P, CW], QK_DT, name="qstg")
                nc.vector.tensor_copy(stg[:], ps[:, :CW])
                nc.sync.dma_start(
                    qt_d.ap()[oc * P:(oc + 1) * P, sc * CW:(sc + 1) * CW],
                    stg[:])

        # ---- phase K: kT = Wk @ key^T -> resident SBUF [128, OC, T] ----
        wkT = load_wT(wk_d, QK_DT)
        kt = kt_pool.tile([P, OC, T], QK_DT)
        for sc in range(T // CW):
            kT_in = in_transpose_chunk(k_d, sc * CW, CW)
            for oc in range(OC):
                ps = mm_ps.tile([P, TCH], F32, name="pjps", tag="mmps")
                for ic in range(OC):
                    _mm(nc, ps[:, :CW], wkT[:, ic, oc * P:(oc + 1) * P],
                        kT_in[:, ic, :], QK_DT,
                        start=(ic == 0), stop=(ic == OC - 1))
                nc.vector.tensor_copy(
                    kt[:, oc, sc * CW:(sc + 1) * CW], ps[:, :CW])

        # ---- phase V: value -> resident bf16 [128(t_loc), 16(t_blk), D] ----
        wvT = load_wT(wv_d, VP_DT)
        val = val_pool.tile([P, T // P, D], AV_DT)
        for tb in range(T // P):
            stg = stage.tile([P, D], F32, name="stage", tag="stage")
            nc.sync.dma_start(stg[:], v_d.ap()[tb * P:(tb + 1) * P, :])
            nc.vector.tensor_copy(val[:, tb, :], stg[:])

        # ---- strip loop ----
        for si in range(NSTRIP):
            s0 = si * P
            nch = (s0 + P + TCH - 1) // TCH   # t-chunks covering [0, s0+128)
            ntb = si + 1                      # 128-wide t-blocks in play

            qts = qts_pool.tile([P, OC, P], QK_DT, name="qts")
            nc.sync.dma_start(
                qts[:],
                qt_d.ap()[:, s0:s0 + P].rearrange("(oc p) s -> p oc s", p=P))

            scores = sc_pool.tile([P, S], F32, name="scores")
            # pass A: scores chunks, diag-masked, parked in SBUF
            for c in range(nch):
                ps = mm_ps.tile([P, TCH], F32, name="scps", tag="mmps")
                for oc in range(OC):
                    _mm(nc, ps[:], qts[:, oc, :],
                        kt[:, oc, c * TCH:(c + 1) * TCH], QK_DT,
                        start=(oc == 0), stop=(oc == OC - 1))
                dst = scores[:, c * TCH:(c + 1) * TCH]
                if c < nch - 1:
                    nc.vector.tensor_copy(dst, ps[:])
                else:
                    nc.vector.tensor_add(dst, ps[:], diagmask[:, si % 4, :])
            rowmax = st_pool.tile([P, 1], F32, name="rowmax", tag="st")
            nc.vector.reduce_max(rowmax[:], scores[:, :nch * TCH],
                                 axis=mybir.AxisListType.X)
            negm = st_pool.tile([P, 1], F32, name="negm", tag="st")
            nc.vector.tensor_scalar_mul(negm[:], rowmax[:], -SCALE)

            # pass B: exp (+row sums), transpose to attnT
            attnT = at_pool.tile([P, NSTRIP, P], AV_DT, name="attnT")
            partials = []
            for c in range(nch):
                expc = exp_pool.tile([P, TCH], AV_DT, name="expc")
                part = st_pool.tile([P, 1], F32, name="part", tag="st")
                nc.scalar.activation(expc[:], scores[:, c * TCH:(c + 1) * TCH],
                                     mybir.ActivationFunctionType.Exp,
                                     bias=negm[:], scale=SCALE,
                                     accum_out=part[:])
                partials.append(part)
                nblk = min(4, ntb - 4 * c)    # skip all-zero blocks past diag
                ps = mm_ps.tile([P, TCH], AV_DT, name="tps2", tag="mmps")
                for g in range(nblk):
                    nc.tensor.transpose(ps[:, g * P:(g + 1) * P],
                                        expc[:, g * P:(g + 1) * P],
                                        ident_bf[:])
                nc.vector.tensor_copy(
                    attnT[:, 4 * c:4 * c + nblk, :],
                    ps[:, :nblk * P].rearrange("p (c s) -> p c s", c=nblk))
            rowsum = st_pool.tile([P, 1], F32, name="rowsum", tag="st")
            if len(partials) == 1:
                nc.vector.tensor_copy(rowsum[:], partials[0][:])
            else:
                nc.vector.tensor_add(rowsum[:], partials[0][:], partials[1][:])
                for part in partials[2:]:
                    nc.vector.tensor_add(rowsum[:], rowsum[:], part[:])

            # AV: ctxT[i, s] = sum_t value[t, i] * attnT[t, s]
            # one accumulation group per PSUM bank: start clears has_written
            # for the WHOLE bank, so only the first matmul in each bank may
            # set start=True (per-element bits handle first-write-overwrite)
            cps = ctx_ps.tile([P, OC, P], F32, name="ctxps")
            for tb in range(ntb):
                for ic in range(OC):
                    nc.tensor.matmul(cps[:, ic, :],
                                     val[:, tb, ic * P:(ic + 1) * P],
                                     attnT[:, tb, :],
                                     start=(tb == 0 and ic % 4 == 0),
                                     stop=(tb == ntb - 1 and ic % 4 == 3),
                                     skip_group_check=True)
            ctxsb = ctx_pool.tile([P, OC, P], VP_DT, name="ctxsb")
            nc.vector.tensor_copy(ctxsb[:], cps[:])

            # V-projection + normalization, out[s, :] strip
            recip = st_pool.tile([P, 1], F32, name="recip", tag="st")
            nc.vector.reciprocal(recip[:], rowsum[:])
            for dc in range(2):
                ps = mm_ps.tile([P, TCH], F32, name="vops", tag="mmps")
                for ic in range(OC):
                    _mm(nc, ps[:], ctxsb[:, ic, :],
                        wvT[:, ic, dc * TCH:(dc + 1) * TCH], VP_DT,
                        start=(ic == 0), stop=(ic == OC - 1))
                ob = ob_pool.tile([P, TCH], F32, name="ob")
                nc.vector.tensor_scalar_mul(ob[:], ps[:], recip[:])
                nc.sync.dma_start(
                    out_d.ap()[s0:s0 + P, dc * TCH:(dc + 1) * TCH], ob[:])

    nc.finalize()
    return nc


_NC_CACHE = None


def kernel(**inputs):
    global _NC_CACHE
    if _NC_CACHE is None:
        _NC_CACHE = build_nc()
    nc = _NC_CACHE
    query = np.ascontiguousarray(inputs["query"], dtype=np.float32)
    key = np.ascontiguousarray(inputs["key"], dtype=np.float32)
    value = np.ascontiguousarray(inputs["value"], dtype=np.float32)
    Wq = np.ascontiguousarray(inputs["Wq"], dtype=np.float32)
    Wk = np.ascontiguousarray(inputs["Wk"], dtype=np.float32)
    Wv = np.ascontiguousarray(inputs["Wv"], dtype=np.float32)
    in_maps = [
        {"query": query[i], "key": key[i], "value": value[i],
         "Wq": Wq, "Wk": Wk, "Wv": Wv}
        for i in range(N)
    ]
    res = run_bass_kernel_spmd(nc, in_maps, core_ids=list(range(N)))
    return np.stack([res.results[i]["out"] for i in range(N)], axis=0)


# revision 8
# speedup vs baseline: 1.1945x; 1.1945x over previous
"""Causal attention layer on 8 TRN2 NeuronCores, data-parallel over batch.

Per-core problem (batch element n = core id):
    q = query @ Wq.T ; k = key @ Wk.T              (f32r matmuls)
    scores[s,t] = q[s]·k[t]  for t <= s            (f32r)
    attn = softmax(32 * scores)  (the +1 additive mask cancels in softmax;
                                  -inf masking == skipping t > s)
    ctxT[i,s] = sum_t value[t,i] * attn[s,t]       (bf16)
    out[s,:]  = (ctxT.T @ Wv.T) / rowsum           (f32r, normalization folded)

Layouts: qT/kT are built as [D, S] via PE transposes of the inputs and
weights so every matmul contracts along partitions without DMA transposes.
"""
import numpy as np
from contextlib import ExitStack

import concourse.bass as bass
import concourse.tile as tile
from concourse import bacc, mybir
from concourse.bass_utils import run_bass_kernel_spmd
from concourse.masks import make_identity

F32 = mybir.dt.float32
F32R = mybir.dt.float32r
BF16 = mybir.dt.bfloat16

N, S, T, D = 8, 2048, 2048, 1024
P = 128
NSTRIP = S // P          # 16 query strips
TCH = 512                # t-chunk for score matmuls
OC = D // P              # 8 chunks of the projection/feature dim
SCALE = float(np.sqrt(np.float32(D)))  # 32.0
NEG = -1.0e30

# dtype knobs (QK path needs >= f32r precision; see noise_sim.py)
QK_DT = F32R             # q/k projections + scores matmuls
AV_DT = BF16             # attn weights + value contraction
VP_DT = F32R             # final (attn@value) @ Wv.T projection


def _mm(nc, out, lhsT, rhs, dt, **kw):
    nc.tensor.matmul(out, lhsT.bitcast(dt), rhs.bitcast(dt), **kw)


PHASE_MARKS = []


def _mark(nc, label):
    n = nc.next_id()  # consumes one id; records build position
    PHASE_MARKS.append((label, n))


def build_nc():
    PHASE_MARKS.clear()
    nc = bacc.Bacc("TRN2", target_bir_lowering=False, debug=False,
                   enable_asserts=False)
    q_d = nc.dram_tensor("query", [S, D], F32, kind="ExternalInput")
    k_d = nc.dram_tensor("key", [T, D], F32, kind="ExternalInput")
    v_d = nc.dram_tensor("value", [T, D], F32, kind="ExternalInput")
    wq_d = nc.dram_tensor("Wq", [D, D], F32, kind="ExternalInput")
    wk_d = nc.dram_tensor("Wk", [D, D], F32, kind="ExternalInput")
    wv_d = nc.dram_tensor("Wv", [D, D], F32, kind="ExternalInput")
    out_d = nc.dram_tensor("out", [S, D], F32, kind="ExternalOutput")
    qt_d = nc.dram_tensor("qt_scratch", [D, S], QK_DT)  # internal scratch

    with tile.TileContext(nc) as tc, ExitStack() as ctx:
        const = ctx.enter_context(tc.tile_pool(name="const", bufs=1))
        wt_pool = ctx.enter_context(tc.tile_pool(name="wt", bufs=1))
        kt_pool = ctx.enter_context(tc.tile_pool(name="kt", bufs=1))
        val_pool = ctx.enter_context(tc.tile_pool(name="val", bufs=1))
        stage = ctx.enter_context(tc.tile_pool(name="stage", bufs=2))
        int_pool = ctx.enter_context(tc.tile_pool(name="inT", bufs=1))
        qts_pool = ctx.enter_context(tc.tile_pool(name="qts", bufs=2))
        qstg_pool = ctx.enter_context(tc.tile_pool(name="qstg", bufs=3))
        sc_pool = ctx.enter_context(tc.tile_pool(name="scores", bufs=2))
        exp_pool = ctx.enter_context(tc.tile_pool(name="exp", bufs=2))
        at_pool = ctx.enter_context(tc.tile_pool(name="attnT", bufs=1))
        ctx_pool = ctx.enter_context(tc.tile_pool(name="ctxsb", bufs=2))
        ctxn_pool = ctx.enter_context(tc.tile_pool(name="ctxn", bufs=1))
        ob_pool = ctx.enter_context(tc.tile_pool(name="outb", bufs=2))
        st_pool = ctx.enter_context(tc.tile_pool(name="stats", bufs=32))
        mm_ps = ctx.enter_context(tc.tile_pool(name="mmps", bufs=4, space="PSUM"))
        ctx_ps = ctx.enter_context(tc.tile_pool(name="ctxps", bufs=2, space="PSUM"))

        ident = const.tile([P, P], F32)
        make_identity(nc, ident)
        ident_bf = const.tile([P, P], BF16)
        nc.vector.tensor_copy(ident_bf[:], ident[:])

        # Additive diag masks for the 4 possible strip positions inside a
        # 512-wide t-chunk: mask[j][x, y] = 0 if y <= j*128 + x else -1e30
        diagmask = const.tile([P, 4, TCH], BF16)
        nc.gpsimd.memset(diagmask[:], 0.0)
        for j in range(4):
            nc.gpsimd.affine_select(
                out=diagmask[:, j, :], in_=diagmask[:, j, :],
                compare_op=mybir.AluOpType.is_ge, fill=NEG,
                base=j * P, channel_multiplier=1, pattern=[[-1, TCH]])

        def load_wT(w_dram, dt):
            """[D,D] weight [o,i] -> SBUF [128(i_loc), OC(i_chunk), D(o)]."""
            wt = wt_pool.tile([P, OC, D], dt, name="wT", tag="wT")
            for r in range(OC):              # o-strip
                stg = stage.tile([P, D], F32, name="stage", tag="stage")
                nc.sync.dma_start(stg[:], w_dram.ap()[r * P:(r + 1) * P, :])
                for g in range(0, OC, 4):
                    ps = mm_ps.tile([P, TCH], F32, name="tps", tag="mmps")
                    for c in range(4):
                        nc.tensor.transpose(
                            ps[:, c * P:(c + 1) * P],
                            stg[:, (g + c) * P:(g + c + 1) * P], ident[:])
                    nc.vector.tensor_copy(
                        wt[:, g:g + 4, r * P:(r + 1) * P],
                        ps.rearrange("p (c s) -> p c s", c=4))
            return wt

        def in_transpose_chunk(x_dram, c0, width):
            """x[c0:c0+width, :D] -> xT tile [128(i_loc), OC, width]."""
            xt = int_pool.tile([P, OC, 2 * P], QK_DT, name="inT", tag="inT")
            for sl in range(width // P):
                stg = stage.tile([P, D], F32, name="stage", tag="stage")
                nc.sync.dma_start(
                    stg[:], x_dram.ap()[c0 + sl * P:c0 + (sl + 1) * P, :])
                for g in range(0, OC, 4):
                    ps = mm_ps.tile([P, TCH], F32, name="tps", tag="mmps")
                    for c in range(4):
                        nc.tensor.transpose(
                            ps[:, c * P:(c + 1) * P],
                            stg[:, (g + c) * P:(g + c + 1) * P], ident[:])
                    nc.vector.tensor_copy(
                        xt[:, g:g + 4, sl * P:(sl + 1) * P],
                        ps.rearrange("p (c s) -> p c s", c=4))
            return xt[:, :, :width]

        _mark(nc, 'phaseWq')
        # ---- phase Q: qT = Wq @ query^T -> DRAM scratch [D, S] ----
        wqT = load_wT(wq_d, QK_DT)
        CW = 256                      # projection chunk width (f32r wants >=256)
        _mark(nc, 'phaseQ')
        for sc in range(S // CW):
            qT = in_transpose_chunk(q_d, sc * CW, CW)
            for oc in range(OC):
                ps = mm_ps.tile([P, TCH], F32, name="pjps", tag="mmps")
                for ic in range(OC):
                    _mm(nc, ps[:, :CW], wqT[:, ic, oc * P:(oc + 1) * P],
                        qT[:, ic, :], QK_DT,
                        start=(ic == 0), stop=(ic == OC - 1))
                stg = qstg_pool.tile([P, CW], QK_DT, name="qstg")
                nc.vector.tensor_copy(stg[:], ps[:, :CW])
                nc.sync.dma_start(
                    qt_d.ap()[oc * P:(oc + 1) * P, sc * CW:(sc + 1) * CW],
                    stg[:])

        _mark(nc, 'phaseWk')
        # ---- phase K: kT = Wk @ key^T -> resident SBUF [128, OC, T] ----
        wkT = load_wT(wk_d, QK_DT)
        kt = kt_pool.tile([P, OC, T], QK_DT)
        _mark(nc, 'phaseK')
        for sc in range(T // CW):
            kT_in = in_transpose_chunk(k_d, sc * CW, CW)
            for oc in range(OC):
                ps = mm_ps.tile([P, TCH], F32, name="pjps", tag="mmps")
                for ic in range(OC):
                    _mm(nc, ps[:, :CW], wkT[:, ic, oc * P:(oc + 1) * P],
                        kT_in[:, ic, :], QK_DT,
                        start=(ic == 0), stop=(ic == OC - 1))
                nc.vector.tensor_copy(
                    kt[:, oc, sc * CW:(sc + 1) * CW], ps[:, :CW])

        _mark(nc, 'phaseWv')
        # ---- phase V: value -> resident bf16 [128(t_loc), 16(t_blk), D] ----
        wvT = load_wT(wv_d, VP_DT)
        val = val_pool.tile([P, T // P, D], AV_DT)
        for tb in range(T // P):
            stg = stage.tile([P, D], F32, name="stage", tag="stage")
            nc.sync.dma_start(stg[:], v_d.ap()[tb * P:(tb + 1) * P, :])
            nc.vector.tensor_copy(val[:, tb, :], stg[:])

        # ---- strip loop (software-pipelined) ----
        # pass A of strip si+1 is emitted BEFORE pass B of strip si so the
        # PE has score matmuls to chew on while the (serial) softmax chain
        # of strip si runs on DVE/ACT.
        state = {}

        def pass_a(si):
            _mark(nc, f'strip{si}')
            s0 = si * P
            nch = (s0 + P + TCH - 1) // TCH   # t-chunks covering [0, s0+128)
            qts = qts_pool.tile([P, OC, P], QK_DT, name="qts")
            nc.sync.dma_start(
                qts[:],
                qt_d.ap()[:, s0:s0 + P].rearrange("(oc p) s -> p oc s", p=P))

            scores = sc_pool.tile([P, S], F32, name="scores")
            # scores chunks, diag-masked, parked in SBUF
            for c in range(nch):
                ps = mm_ps.tile([P, TCH], F32, name="scps", tag="mmps")
                for oc in range(OC):
                    _mm(nc, ps[:], qts[:, oc, :],
                        kt[:, oc, c * TCH:(c + 1) * TCH], QK_DT,
                        start=(oc == 0), stop=(oc == OC - 1))
                dst = scores[:, c * TCH:(c + 1) * TCH]
                if c < nch - 1:
                    nc.vector.tensor_copy(dst, ps[:])
                else:
                    nc.vector.tensor_add(dst, ps[:], diagmask[:, si % 4, :])
            rowmax = st_pool.tile([P, 1], F32, name="rowmax", tag="st")
            nc.vector.reduce_max(rowmax[:], scores[:, :nch * TCH],
                                 axis=mybir.AxisListType.X)
            negm = st_pool.tile([P, 1], F32, name="negm", tag="st")
            nc.vector.tensor_scalar_mul(negm[:], rowmax[:], -SCALE)
            state[si] = (scores, negm)

        def pass_b(si):
            s0 = si * P
            nch = (s0 + P + TCH - 1) // TCH
            ntb = si + 1                      # 128-wide t-blocks in play
            scores, negm = state.pop(si)

            # exp (+row sums), transpose to attnT
            attnT = at_pool.tile([P, NSTRIP, P], AV_DT, name="attnT")
            partials = []
            for c in range(nch):
                expc = exp_pool.tile([P, TCH], AV_DT, name="expc")
                part = st_pool.tile([P, 1], F32, name="part", tag="st")
                nc.scalar.activation(expc[:], scores[:, c * TCH:(c + 1) * TCH],
                                     mybir.ActivationFunctionType.Exp,
                                     bias=negm[:], scale=SCALE,
                                     accum_out=part[:])
                partials.append(part)
                nblk = min(4, ntb - 4 * c)    # skip all-zero blocks past diag
                ps = mm_ps.tile([P, TCH], AV_DT, name="tps2", tag="mmps")
                for g in range(nblk):
                    nc.tensor.transpose(ps[:, g * P:(g + 1) * P],
                                        expc[:, g * P:(g + 1) * P],
                                        ident_bf[:])
                nc.vector.tensor_copy(
                    attnT[:, 4 * c:4 * c + nblk, :],
                    ps[:, :nblk * P].rearrange("p (c s) -> p c s", c=nblk))
            rowsum = st_pool.tile([P, 1], F32, name="rowsum", tag="st")
            if len(partials) == 1:
                nc.vector.tensor_copy(rowsum[:], partials[0][:])
            else:
                nc.vector.tensor_add(rowsum[:], partials[0][:], partials[1][:])
                for part in partials[2:]:
                    nc.vector.tensor_add(rowsum[:], rowsum[:], part[:])

            # AV: ctx[s, i] = sum_t attn[s, t] * value[t, i]
            # (lhsT = attnT block, rhs = value row-block, N=512 moving)
            cps = ctx_ps.tile([P, 2, TCH], F32, name="ctxps")
            for tb in range(ntb):
                for ih in range(2):
                    nc.tensor.matmul(cps[:, ih, :],
                                     attnT[:, tb, :],
                                     val[:, tb, ih * TCH:(ih + 1) * TCH],
                                     start=(tb == 0), stop=(tb == ntb - 1))
            ctxn = ctxn_pool.tile([P, 2, TCH], F32, name="ctxn")
            nc.vector.tensor_copy(ctxn[:], cps[:])
            # transpose ctx -> ctxT [i, s] for the V projection
            ctxsb = ctx_pool.tile([P, OC, P], VP_DT, name="ctxsb")
            for g in range(2):
                tp = mm_ps.tile([P, TCH], F32, name="ctp", tag="mmps")
                for c in range(4):
                    nc.tensor.transpose(
                        tp[:, c * P:(c + 1) * P],
                        ctxn[:, g, c * P:(c + 1) * P], ident[:])
                nc.vector.tensor_copy(
                    ctxsb[:, 4 * g:4 * g + 4, :],
                    tp.rearrange("p (c s) -> p c s", c=4))

            # V-projection + normalization, out[s, :] strip
            recip = st_pool.tile([P, 1], F32, name="recip", tag="st")
            nc.vector.reciprocal(recip[:], rowsum[:])
            for dc in range(2):
                ps = mm_ps.tile([P, TCH], F32, name="vops", tag="mmps")
                for ic in range(OC):
                    _mm(nc, ps[:], ctxsb[:, ic, :],
                        wvT[:, ic, dc * TCH:(dc + 1) * TCH], VP_DT,
                        start=(ic == 0), stop=(ic == OC - 1))
                ob = ob_pool.tile([P, TCH], F32, name="ob")
                nc.vector.tensor_scalar_mul(ob[:], ps[:], recip[:])
                nc.sync.dma_start(
                    out_d.ap()[s0:s0 + P, dc * TCH:(dc + 1) * TCH], ob[:])

        pass_a(0)
        for si in range(NSTRIP):
            if si + 1 < NSTRIP:
                pass_a(si + 1)
            pass_b(si)

    _mark(nc, 'end')
    nc.finalize()
    return nc


_NC_CACHE = None


def kernel(**inputs):
    global _NC_CACHE
    if _NC_CACHE is None:
        _NC_CACHE = build_nc()
    nc = _NC_CACHE
    query = np.ascontiguousarray(inputs["query"], dtype=np.float32)
    key = np.ascontiguousarray(inputs["key"], dtype=np.float32)
    value = np.ascontiguousarray(inputs["value"], dtype=np.float32)
    Wq = np.ascontiguousarray(inputs["Wq"], dtype=np.float32)
    Wk = np.ascontiguousarray(inputs["Wk"], dtype=np.float32)
    Wv = np.ascontiguousarray(inputs["Wv"], dtype=np.float32)
    in_maps = [
        {"query": query[i], "key": key[i], "value": value[i],
         "Wq": Wq, "Wk": Wk, "Wv": Wv}
        for i in range(N)
    ]
    res = run_bass_kernel_spmd(nc, in_maps, core_ids=list(range(N)))
    return np.stack([res.results[i]["out"] for i in range(N)], axis=0)
